# revision 13
# baseline (speedup 1.0000x reference)
"""BiLSTM-CRF NLL loss kernel for 8 Trainium2 NeuronCores (pure batch data-parallel).

Self-contained: hardcodes all shapes. Strategy per core (32 of 256 sequences):
  1. Embedding gather (indirect DMA, bf16 table) -> PE transpose -> augmented
     input-projection GEMM gx = W_aug @ [emb; 1; 1-m] (bias + bwd-mask folded in).
  2. Backward LSTM pass (global t descending), then forward pass. Weights
     stationary bf16 matmuls accumulate onto gx preloaded in PSUM; gates via
     tanh only (sigmoid(x) = 0.5*tanh(x/2)+0.5) so one ACT table set serves
     the whole kernel (exp_and_others: tanh+exp+copy).
  3. Bulk fc GEMM -> feats = mask*(W_fc@h) + b_fc -> exp(feats).
  4. CRF forward in scaled probability space: A <- (P @ A) * ef_t with
     P = exp(trans) stationary on PE; renormalize every 6 steps, log(Z)
     summed in bulk at the end (natural_log set, one switch).
  5. Gold score from host-built integer histograms/one-hots via TTR + tiny
     PE reductions. Output: per-core sum of (fwd - gold); host divides by B.
"""

import numpy as np

import concourse.bacc as bacc
import concourse.bass as bass
import concourse.mybir as mybir
import concourse.tile as tile
from concourse import bass_utils

B, T, E, H2, V, NT = 256, 192, 300, 256, 11626, 13
H = H2 // 2          # 128
G4 = 4 * H           # 512
START, STOP = 0, 10
NCORES = 8
BC = B // NCORES     # 32 sequences per core
TOK = BC * T         # 6144 tokens per core, t-major: tok = t*BC + b
KAUG = E + 2         # emb dims + ones row + (1-m) row
KCH = [(0, 128), (128, 256), (256, KAUG)]   # K chunks of augmented GEMM
NCHUNK = 512         # gx GEMM moving free dim
RENORM = 6           # CRF renorm period
NREN = T // RENORM   # 32 renorm events

FP32 = mybir.dt.float32
BF16 = mybir.dt.bfloat16
I32 = mybir.dt.int32
AF = mybir.ActivationFunctionType
ALU = mybir.AluOpType

_PROGRAM_CACHE = {}


def _emit(tc, io):
    nc = tc.nc
    ident = io["ident"]; sent = io["sent"]; embtab = io["embtab"]
    waug = io["waug"]; whh = io["whh"]; wfc = io["wfc"]; bfc = io["bfc"]
    transT = io["transT"]; transflat = io["transflat"]; m1 = io["m1"]
    aux = io["aux"]; oh = io["oh"]; pc = io["pc"]; out = io["out"]

    import contextlib
    ctx = contextlib.ExitStack()
    with ctx:
        consts = ctx.enter_context(tc.tile_pool(name="consts", bufs=1))

        # ---------- constants into SBUF ----------
        ident_sb = consts.tile([128, 128], BF16)
        nc.sync.dma_start(out=ident_sb[:], in_=ident[:])
        sent_sb = consts.tile([128, TOK // 128], I32)
        nc.sync.dma_start(out=sent_sb[:], in_=sent.rearrange("(c p) -> p c", p=128))
        waug_sb = [consts.tile([k1 - k0, 2 * G4], BF16, name=f"waug{i}") for i, (k0, k1) in enumerate(KCH)]
        for (k0, k1), t_ in zip(KCH, waug_sb):
            nc.sync.dma_start(out=t_[:], in_=waug[k0:k1, :])
        whh_sb = consts.tile([H, 2 * G4], BF16)
        nc.sync.dma_start(out=whh_sb[:], in_=whh[:])
        wfc_sb = consts.tile([H, 2 * NT], BF16)  # col blocks: [W_fc h_f-half | h_b-half]
        nc.sync.dma_start(out=wfc_sb[:, 0:NT], in_=wfc[0:H, :])
        nc.sync.dma_start(out=wfc_sb[:, NT:2 * NT], in_=wfc[H:H2, :])
        bfc_sb = consts.tile([NT, 1], FP32)
        nc.sync.dma_start(out=bfc_sb[:], in_=bfc[:])
        transT_sb = consts.tile([NT, NT], FP32)
        nc.sync.dma_start(out=transT_sb[:], in_=transT[:])
        tf_sb = consts.tile([128, 2], FP32)   # transflat split: col0 rows 0:128, col1 rows 0:41
        tfa = transflat[0:128]
        nc.sync.dma_start(out=tf_sb[:, 0:1], in_=bass.AP(tensor=tfa.tensor, offset=tfa.offset, ap=[[1, 128], [1, 1]]))
        tfb = transflat[128:169]
        nc.sync.dma_start(out=tf_sb[0:41, 1:2], in_=bass.AP(tensor=tfb.tensor, offset=tfb.offset, ap=[[1, 41], [1, 1]]))
        pc_sb = consts.tile([128, 2 * BC], FP32)  # PC chunks side by side
        nc.sync.dma_start(out=pc_sb[:, 0:BC], in_=pc[0:128, :])
        nc.sync.dma_start(out=pc_sb[0:41, BC:2 * BC], in_=pc[128:169, :])
        ones13_sb = consts.tile([NT, 1], FP32)
        nc.vector.memset(ones13_sb[:], 1.0)
        onesrow_sb = consts.tile([1, NT], FP32)
        nc.vector.memset(onesrow_sb[:], 1.0)
        # P^T = exp(transT) (stationary CRF matrix, lhsT form), also used for PSTOP col
        pt_sb = consts.tile([NT, NT], FP32)
        nc.scalar.activation(pt_sb[:], transT_sb[:], AF.Exp)

        # ---------- phase 1: gather -> transpose -> gx GEMMs ----------
        hallp = ctx.enter_context(tc.tile_pool(name="hallp", bufs=1))
        h_all = [hallp.tile([H, TOK], BF16, name=f"hall{i}") for i in range(2)]
        gxdp = tc.alloc_tile_pool(name="gxdp", bufs=1)
        gxd = [gxdp.tile([128, 4 * TOK], BF16, name=f"gxd{i}") for i in range(2)]
        embp = tc.alloc_tile_pool(name="embp", bufs=1)
        embT = [embp.tile([k1 - k0, TOK], BF16, name=f"embT{i}") for i, (k0, k1) in enumerate(KCH)]  # augmented emb^T
        nc.sync.dma_start(out=embT[2][E - 256:E - 254, :], in_=aux[:])  # [ones; 1-m] rows 44,45

        gpool = tc.alloc_tile_pool(name="gather", bufs=4)
        tpsum = tc.alloc_tile_pool(name="tpsum", bufs=4, space="PSUM")
        nchunks = TOK // 128
        for c in range(nchunks):
            embg = gpool.tile([128, E], BF16)
            nc.gpsimd.indirect_dma_start(
                out=embg[:], out_offset=None, in_=embtab[:],
                in_offset=bass.IndirectOffsetOnAxis(ap=sent_sb[:, c:c + 1], axis=0),
            )
            for ki, (k0, k1) in enumerate(KCH):
                kw = min(k1, E) - k0  # 128,128,44 data rows
                tp = tpsum.tile([128, 128], BF16)
                nc.tensor.transpose(tp[0:kw, :], embg[:, k0:k0 + kw], ident_sb[:])
                eng = nc.scalar if (c + ki) % 2 == 0 else nc.vector
                if eng is nc.scalar:
                    nc.scalar.copy(embT[ki][0:kw, c * 128:(c + 1) * 128], tp[0:kw, :])
                else:
                    nc.vector.tensor_copy(embT[ki][0:kw, c * 128:(c + 1) * 128], tp[0:kw, :])

        tpsum.release()
        # gx GEMMs: gxd[d] layout [128, 4*TOK], cols g*TOK + tok
        gxp = tc.alloc_tile_pool(name="gxp", bufs=3, space="PSUM")
        for d in range(2):
            nrange = range(TOK // NCHUNK)
            if d == 1:
                nrange = reversed(list(nrange))  # bwd gx in descending t order
            for n in nrange:
                for g in range(4):
                    mm = gxp.tile([128, NCHUNK], FP32)
                    for ki, (k0, k1) in enumerate(KCH):
                        nc.tensor.matmul(
                            mm[:],
                            waug_sb[ki][:, d * G4 + g * H: d * G4 + (g + 1) * H],
                            embT[ki][:, n * NCHUNK:(n + 1) * NCHUNK],
                            start=(ki == 0), stop=(ki == 2),
                        )
                    dst = gxd[d][:, g * TOK + n * NCHUNK: g * TOK + (n + 1) * NCHUNK]
                    if (n + g) % 2 == 0:
                        nc.scalar.copy(dst, mm[:])
                    else:
                        nc.vector.tensor_copy(dst, mm[:])

        gxp.release()
        gpool.release()
        embp.release()

        # ---------- phase 2: LSTM passes ----------
        hinit = consts.tile([H, BC], BF16)
        nc.vector.memset(hinit[:], 0.0)
        cinit = consts.tile([H, BC], FP32)
        nc.vector.memset(cinit[:], 0.0)

        lpools = {
            "gates": tc.alloc_tile_pool(name="lgates", bufs=2, space="PSUM"),
            "tio": tc.alloc_tile_pool(name="ltio", bufs=2),
            "tg": tc.alloc_tile_pool(name="ltg", bufs=2),
            "sig": tc.alloc_tile_pool(name="lsig", bufs=2),
            "ab": tc.alloc_tile_pool(name="lab", bufs=2),
            "c": tc.alloc_tile_pool(name="lc", bufs=2),
            "tc": tc.alloc_tile_pool(name="ltc", bufs=2),
        }

        def lstm_step(d, t, h_prev_ap, c_prev_ap):
            # gates PSUM [H, 128]: cols g*BC..(g+1)*BC = gate g (order i,f,o,g)
            gates = lpools["gates"].tile([H, 4 * BC], FP32)
            # preload gx (4 strided gate blocks -> contiguous psum) on ACT
            gx_src = bass.AP(
                tensor=gxd[d].tensor, offset=gxd[d][:, t * BC].offset,
                ap=[gxd[d].ap[0], [TOK, 4], [1, BC]],
            )
            nc.scalar.copy(gates[:], gx_src)
            for g in range(4):
                nc.tensor.matmul(
                    gates[:, g * BC:(g + 1) * BC],
                    whh_sb[:, d * G4 + g * H: d * G4 + (g + 1) * H],
                    h_prev_ap,
                    start=False, stop=True, skip_group_check=True,
                )
            tio = lpools["tio"].tile([H, 3 * BC], FP32)
            nc.scalar.activation(tio[:], gates[:, 0:3 * BC], AF.Tanh, scale=0.5)
            tg = lpools["tg"].tile([H, BC], FP32)
            nc.scalar.activation(tg[:], gates[:, 3 * BC:4 * BC], AF.Tanh)
            sig = lpools["sig"].tile([H, 3 * BC], FP32)  # si|sf|so
            for k in range(3):
                nc.gpsimd.tensor_scalar(
                    sig[:, k * BC:(k + 1) * BC], tio[:, k * BC:(k + 1) * BC],
                    0.5, 0.5, ALU.mult, ALU.add,
                )
            a_ = lpools["ab"].tile([H, BC], FP32, tag="a")
            nc.vector.tensor_tensor(a_[:], sig[:, 0:BC], tg[:], ALU.mult)
            b_ = lpools["ab"].tile([H, BC], FP32, tag="b")
            nc.vector.tensor_tensor(b_[:], sig[:, BC:2 * BC], c_prev_ap, ALU.mult)
            c_ = lpools["c"].tile([H, BC], FP32)
            nc.vector.tensor_tensor(c_[:], a_[:], b_[:], ALU.add)
            tc_ = lpools["tc"].tile([H, BC], FP32)
            nc.scalar.activation(tc_[:], c_[:], AF.Tanh)
            hdst = h_all[d][:, t * BC:(t + 1) * BC]
            nc.vector.tensor_tensor(hdst, sig[:, 2 * BC:3 * BC], tc_[:], ALU.mult)
            return hdst, c_[:]

        # backward pass (d=1): global t descending
        hp, cp = hinit[:], cinit[:]
        for s in range(T):
            t = T - 1 - s
            hp, cp = lstm_step(1, t, hp, cp)
        # forward pass (d=0): ascending
        hp, cp = hinit[:], cinit[:]
        for t in range(T):
            hp, cp = lstm_step(0, t, hp, cp)

        for pname in ["tc", "c", "ab", "sig", "tg", "tio"]:
            lpools[pname].release()
        lpools["gates"].release()
        gxdp.release()

        # ---------- phase 3: fc GEMM -> feats -> ef ----------
        late = ctx.enter_context(tc.tile_pool(name="late", bufs=1))
        feats = late.tile([NT, TOK], BF16)
        ef = late.tile([NT, TOK], BF16)
        m13_sb = late.tile([NT, TOK], BF16)
        nc.sync.dma_start(
            out=m13_sb[:],
            in_=bass.AP(tensor=m1.tensor, offset=m1.offset, ap=[[0, NT]] + m1.ap),
        )
        oh_sb = late.tile([NT, TOK], BF16)
        nc.sync.dma_start(out=oh_sb[:], in_=oh[:])
        fcp = tc.alloc_tile_pool(name="fcp", bufs=3, space="PSUM")
        fctmp = ctx.enter_context(tc.tile_pool(name="fctmp", bufs=3))
        for n in range(TOK // NCHUNK):
            mm = fcp.tile([NT, NCHUNK], FP32)
            cols = slice(n * NCHUNK, (n + 1) * NCHUNK)
            nc.tensor.matmul(mm[:], wfc_sb[:, 0:NT], h_all[0][:, cols], start=True, stop=False)
            nc.tensor.matmul(mm[:], wfc_sb[:, NT:2 * NT], h_all[1][:, cols], start=False, stop=True)
            msk = fctmp.tile([NT, NCHUNK], FP32)
            nc.vector.tensor_tensor(msk[:], mm[:], m13_sb[:, cols], ALU.mult)
            nc.scalar.activation(feats[:, cols], msk[:], AF.Identity, bias=bfc_sb[:, 0:1])
            nc.scalar.activation(ef[:, cols], msk[:], AF.Exp, bias=bfc_sb[:, 0:1])

        # ---------- phase 4: CRF forward recursion ----------
        fcp.release()
        apool = ctx.enter_context(tc.tile_pool(name="apool", bufs=3))
        crfp = ctx.enter_context(tc.tile_pool(name="crfp", bufs=2, space="PSUM"))
        zbuf = late.tile([1, NREN * BC], FP32)
        zr = ctx.enter_context(tc.tile_pool(name="zr", bufs=2))
        A = apool.tile([NT, BC], FP32, tag="A")
        nc.vector.memset(A[:], 0.0)
        nc.vector.memset(A[START:START + 1, :], 1.0)
        for t in range(T):
            r = crfp.tile([NT, BC], FP32, tag="r")
            nc.tensor.matmul(r[:], pt_sb[:], A[:], start=True, stop=True)
            A = apool.tile([NT, BC], FP32, tag="A")
            nc.vector.tensor_tensor(A[:], r[:], ef[:, t * BC:(t + 1) * BC], ALU.mult)
            if (t + 1) % RENORM == 0:
                k = (t + 1) // RENORM - 1
                zrow = crfp.tile([1, BC], FP32, tag="zrow")
                nc.tensor.matmul(zrow[:], ones13_sb[:], A[:], start=True, stop=True)
                nc.scalar.copy(zbuf[:, k * BC:(k + 1) * BC], zrow[:])
                zrec = zr.tile([1, BC], FP32)
                nc.vector.reciprocal(zrec[:], zrow[:])
                zbc = crfp.tile([NT, BC], FP32, tag="zbc")
                nc.tensor.matmul(zbc[:], onesrow_sb[:], zrec[:], start=True, stop=True)
                A2 = apool.tile([NT, BC], FP32, tag="A")
                nc.vector.tensor_tensor(A2[:], A[:], zbc[:], ALU.mult)
                A = A2

        # ---------- phase 5: finals ----------
        fin = ctx.enter_context(tc.tile_pool(name="fin", bufs=1))
        finp = ctx.enter_context(tc.tile_pool(name="finp", bufs=1, space="PSUM"))
        # emit + trans_sc accumulate into one PSUM [1, BC]
        TQ = T // 4
        emp = ctx.enter_context(tc.tile_pool(name="emp", bufs=2))
        emres = []
        for q in range(4):
            emtmp = emp.tile([NT, TQ * BC], FP32, tag="emtmp")
            cols = slice(q * TQ * BC, (q + 1) * TQ * BC)
            nc.vector.tensor_tensor(emtmp[:], feats[:, cols], oh_sb[:, cols], ALU.mult)
            emq = fin.tile([NT, BC], FP32, name=f"emq{q}", tag=f"emq{q}")
            nc.vector.reduce_sum(
                emq[:], emtmp[:].rearrange("p (t b) -> p b t", t=TQ),
                axis=mybir.AxisListType.X,
            )
            emres.append(emq)
        nc.vector.tensor_tensor(emres[0][:], emres[0][:], emres[1][:], ALU.add)
        nc.vector.tensor_tensor(emres[2][:], emres[2][:], emres[3][:], ALU.add)
        emred = fin.tile([NT, BC], FP32)
        nc.vector.tensor_tensor(emred[:], emres[0][:], emres[2][:], ALU.add)
        gold = finp.tile([1, BC], FP32)
        nc.tensor.matmul(gold[:], tf_sb[:, 0:1], pc_sb[:, 0:BC], start=True, stop=False)
        nc.tensor.matmul(gold[:], tf_sb[0:41, 1:2], pc_sb[0:41, BC:2 * BC], start=False, stop=False)
        nc.tensor.matmul(gold[:], ones13_sb[:], emred[:], start=False, stop=True)
        # fwd score: log(PSTOP . A) + sum_k ln Z_k
        fmm = finp.tile([1, BC], FP32)
        nc.tensor.matmul(fmm[:], pt_sb[:, STOP:STOP + 1], A[:], start=True, stop=True)
        lnz = fin.tile([1, NREN * BC], FP32)
        nc.scalar.activation(lnz[:], zbuf[:], AF.Ln)
        lsum = fin.tile([1, BC], FP32)
        nc.vector.reduce_sum(
            lsum[:], lnz[:].rearrange("p (k b) -> p b k", k=NREN), axis=mybir.AxisListType.X,
        )
        lfin = fin.tile([1, BC], FP32)
        nc.scalar.activation(lfin[:], fmm[:], AF.Ln)
        fwd = fin.tile([1, BC], FP32)
        nc.vector.tensor_tensor(fwd[:], lfin[:], lsum[:], ALU.add)
        nll = fin.tile([1, BC], FP32)
        nc.vector.tensor_tensor(nll[:], fwd[:], gold[:], ALU.subtract)
        tot = fin.tile([1, 1], FP32)
        nc.vector.reduce_sum(tot[:], nll[:], axis=mybir.AxisListType.X)
        nc.sync.dma_start(out=out[:], in_=tot[:])


def build_program():
    if "nc" in _PROGRAM_CACHE:
        return _PROGRAM_CACHE["nc"]
    nc = bacc.Bacc("TRN2", target_bir_lowering=False, debug=False, num_devices=NCORES)
    io = {
        "ident": nc.dram_tensor("ident", [128, 128], BF16, kind="ExternalInput").ap(),
        "sent": nc.dram_tensor("sent", [TOK], I32, kind="ExternalInput").ap(),
        "embtab": nc.dram_tensor("embtab", [V, E], BF16, kind="ExternalInput").ap(),
        "waug": nc.dram_tensor("waug", [KAUG, 2 * G4], BF16, kind="ExternalInput").ap(),
        "whh": nc.dram_tensor("whh", [H, 2 * G4], BF16, kind="ExternalInput").ap(),
        "wfc": nc.dram_tensor("wfc", [H2, NT], BF16, kind="ExternalInput").ap(),
        "bfc": nc.dram_tensor("bfc", [NT, 1], FP32, kind="ExternalInput").ap(),
        "transT": nc.dram_tensor("transT", [NT, NT], FP32, kind="ExternalInput").ap(),
        "transflat": nc.dram_tensor("transflat", [NT * NT], FP32, kind="ExternalInput").ap(),
        "m1": nc.dram_tensor("m1", [TOK], BF16, kind="ExternalInput").ap(),
        "aux": nc.dram_tensor("aux", [2, TOK], BF16, kind="ExternalInput").ap(),
        "oh": nc.dram_tensor("oh", [NT, TOK], BF16, kind="ExternalInput").ap(),
        "pc": nc.dram_tensor("pc", [NT * NT, BC], FP32, kind="ExternalInput").ap(),
        "out": nc.dram_tensor("out", [1, 1], FP32, kind="ExternalOutput").ap(),
    }
    with tile.TileContext(nc) as tc:
        _emit(tc, io)
    nc.compile()
    _PROGRAM_CACHE["nc"] = nc
    return nc


def host_prep(inputs):
    """Build the 8 per-core input maps (host does only index/layout/dtype prep)."""
    import ml_dtypes
    bf16 = ml_dtypes.bfloat16

    sent = np.asarray(inputs["sentence"]).astype(np.int32)      # [B,T]
    seq_len = np.asarray(inputs["seq_len"]).astype(np.int64)
    tags = np.asarray(inputs["tags"]).astype(np.int64)          # [B,T]
    lens = np.clip(seq_len, 1, T)
    mask = (np.arange(T)[None, :] < lens[:, None]).astype(np.float32)  # [B,T]
    embtab_bf = np.ascontiguousarray(np.asarray(inputs["embedding"], np.float32).astype(bf16))

    def reorder(Wx):  # pytorch gate order i,f,g,o -> i,f,o,g
        i, f, g, o = np.split(np.asarray(Wx, np.float32), 4, 0)
        return np.concatenate([i, f, o, g], 0)

    def build_waug(W_ih, bvec, is_bwd):
        Wr = reorder(W_ih)          # [4H, E]
        br = reorder(np.asarray(bvec, np.float32)[:, None])[:, 0]
        Waug = np.zeros((KAUG, G4), np.float32)
        Waug[0:E, :] = Wr.T
        Waug[E, :] = br             # ones row -> bias
        if is_bwd:
            Waug[E + 1, 0:2 * H] = -1e9  # (1-m) row -> i,f preact mask
        return Waug

    waug = np.concatenate(
        [build_waug(inputs["W_ih_f"], inputs["b_f"], False),
         build_waug(inputs["W_ih_b"], inputs["b_b"], True)], axis=1
    ).astype(bf16)                                               # [KAUG, 1024]
    whh = np.concatenate(
        [reorder(inputs["W_hh_f"]).T, reorder(inputs["W_hh_b"]).T], axis=1
    ).astype(np.float32).astype(bf16)                            # [H, 1024]
    wfc = np.ascontiguousarray(np.asarray(inputs["W_fc"], np.float32).T).astype(bf16)  # [H2,NT]
    bfc = np.asarray(inputs["b_fc"], np.float32).reshape(NT, 1)
    trans = np.asarray(inputs["transitions"], np.float32)
    transT = np.ascontiguousarray(trans.T)
    transflat = np.ascontiguousarray(trans.reshape(-1))
    ident = np.eye(128, dtype=np.float32).astype(bf16)

    in_maps = []
    for core in range(NCORES):
        sl = slice(core * BC, (core + 1) * BC)
        s_c, t_c, m_c = sent[sl], tags[sl], mask[sl]             # [BC,T]
        sent_tm = np.ascontiguousarray(s_c.T.reshape(-1)).astype(np.int32)   # tok=t*BC+b
        m_tm = np.ascontiguousarray(m_c.T.reshape(-1)).astype(bf16)
        aux_tm = np.stack([np.ones(TOK, np.float32),
                           1.0 - m_tm.astype(np.float32)]).astype(bf16)
        # one-hot [NT, TOK]
        ohm = np.zeros((NT, TOK), np.float32)
        ttm = t_c.T.reshape(-1)                                  # [TOK]
        ohm[ttm, np.arange(TOK)] = 1.0
        ohm = ohm.astype(bf16)
        # pair-count histogram [169, BC] incl STOP term
        pcm = np.zeros((NT * NT, BC), np.float32)
        text = np.concatenate([np.full((BC, 1), START, np.int64), t_c], 1)
        for b_ in range(BC):
            idx = text[b_, 1:] * NT + text[b_, :-1]
            np.add.at(pcm[:, b_], idx, 1.0)
            pcm[STOP * NT + t_c[b_, -1], b_] += 1.0
        in_maps.append({
            "ident": ident, "sent": sent_tm, "embtab": embtab_bf,
            "waug": waug, "whh": whh, "wfc": wfc, "bfc": bfc,
            "transT": transT, "transflat": transflat,
            "m1": m_tm, "aux": aux_tm, "oh": ohm, "pc": pcm,
        })
    return in_maps


def kernel(**inputs):
    nc = build_program()
    in_maps = host_prep(inputs)
    res = bass_utils.run_bass_kernel_spmd(nc, in_maps, list(range(NCORES)))
    total = sum(float(r["out"][0, 0]) for r in res.results)
    return np.float32(total / B)


# revision 26
# speedup vs baseline: 1.2659x; 1.2659x over previous
"""BiLSTM-CRF NLL loss kernel for 8 Trainium2 NeuronCores (pure batch data-parallel).

Self-contained: hardcodes all shapes. Strategy per core (32 of 256 sequences):
  1. Embedding gather (indirect DMA, bf16 table) -> PE transpose -> augmented
     input-projection GEMM gx = W_aug @ [emb; 1; 1-m] (bias + bwd-mask folded in).
  2. Backward LSTM pass (global t descending), then forward pass. Weights
     stationary bf16 matmuls accumulate onto gx preloaded in PSUM; gates via
     tanh only (sigmoid(x) = 0.5*tanh(x/2)+0.5) so one ACT table set serves
     the whole kernel (exp_and_others: tanh+exp+copy).
  3. Bulk fc GEMM -> feats = mask*(W_fc@h) + b_fc -> exp(feats).
  4. CRF forward in scaled probability space: A <- (P @ A) * ef_t with
     P = exp(trans) stationary on PE; renormalize every 6 steps, log(Z)
     summed in bulk at the end (natural_log set, one switch).
  5. Gold score from host-built integer histograms/one-hots via TTR + tiny
     PE reductions. Output: per-core sum of (fwd - gold); host divides by B.
"""

import numpy as np

import concourse.bacc as bacc
import concourse.bass as bass
import concourse.mybir as mybir
import concourse.tile as tile
from concourse import bass_utils

B, T, E, H2, V, NT = 256, 192, 300, 256, 11626, 13
H = H2 // 2          # 128
G4 = 4 * H           # 512
START, STOP = 0, 10
NCORES = 8
BC = B // NCORES     # 32 sequences per core
TOK = BC * T         # 6144 tokens per core, t-major: tok = t*BC + b
KAUG = E + 2         # emb dims + ones row + (1-m) row
KCH = [(0, 128), (128, 256), (256, KAUG)]   # K chunks of augmented GEMM
NCHUNK = 512         # gx GEMM moving free dim
RENORM = 24          # CRF renorm period (P scaled by e^-CCENT keeps fp32 range safe)
CCENT = 3.0
NREN = T // RENORM - 1  # renorm events (final interval folded into last log)

FP32 = mybir.dt.float32
BF16 = mybir.dt.bfloat16
I32 = mybir.dt.int32
AF = mybir.ActivationFunctionType
ALU = mybir.AluOpType

_PROGRAM_CACHE = {}
PHASE_LIMIT = 5  # 1=gx only, 2=+lstm, 3=+fc, 4=+crf, 5=all (ablation timing knob)


def _emit(tc, io):
    nc = tc.nc
    ident = io["ident"]; sent = io["sent"]; embtab = io["embtab"]
    waug = io["waug"]; whh = io["whh"]; wfc = io["wfc"]; bfc = io["bfc"]
    transT = io["transT"]; transflat = io["transflat"]; m1 = io["m1"]
    aux = io["aux"]; oh = io["oh"]; pc = io["pc"]; out = io["out"]

    import contextlib
    ctx = contextlib.ExitStack()
    with ctx:
        consts = ctx.enter_context(tc.tile_pool(name="consts", bufs=1))

        # ---------- constants into SBUF ----------
        ident_sb = consts.tile([128, 128], BF16)
        nc.sync.dma_start(out=ident_sb[:], in_=ident[:])
        sent_sb = consts.tile([128, TOK // 128], I32)
        nc.sync.dma_start(out=sent_sb[:], in_=sent.rearrange("(c p) -> p c", p=128))
        waug_sb = [consts.tile([k1 - k0, 2 * G4], BF16, name=f"waug{i}") for i, (k0, k1) in enumerate(KCH)]
        for (k0, k1), t_ in zip(KCH, waug_sb):
            nc.sync.dma_start(out=t_[:], in_=waug[k0:k1, :])
        whh_sb = consts.tile([H, 2 * G4], BF16)
        nc.sync.dma_start(out=whh_sb[:], in_=whh[:])
        wfc_sb = consts.tile([H, 2 * NT], BF16)  # col blocks: [W_fc h_f-half | h_b-half]
        nc.sync.dma_start(out=wfc_sb[:, 0:NT], in_=wfc[0:H, :])
        nc.sync.dma_start(out=wfc_sb[:, NT:2 * NT], in_=wfc[H:H2, :])
        bfc_sb = consts.tile([NT, 1], FP32)
        nc.sync.dma_start(out=bfc_sb[:], in_=bfc[:])
        transT_sb = consts.tile([NT, NT], FP32)
        nc.sync.dma_start(out=transT_sb[:], in_=transT[:])
        tf_sb = consts.tile([128, 2], FP32)   # transflat split: col0 rows 0:128, col1 rows 0:41
        tfa = transflat[0:128]
        nc.sync.dma_start(out=tf_sb[:, 0:1], in_=bass.AP(tensor=tfa.tensor, offset=tfa.offset, ap=[[1, 128], [1, 1]]))
        tfb = transflat[128:169]
        nc.sync.dma_start(out=tf_sb[0:41, 1:2], in_=bass.AP(tensor=tfb.tensor, offset=tfb.offset, ap=[[1, 41], [1, 1]]))
        pc_sb = consts.tile([128, 2 * BC], FP32)  # PC chunks side by side
        nc.sync.dma_start(out=pc_sb[:, 0:BC], in_=pc[0:128, :])
        nc.sync.dma_start(out=pc_sb[0:41, BC:2 * BC], in_=pc[128:169, :])
        ones13_sb = consts.tile([NT, 1], FP32)
        nc.vector.memset(ones13_sb[:], 1.0)
        onesrow_sb = consts.tile([1, NT], FP32)
        nc.vector.memset(onesrow_sb[:], 1.0)
        # P^T = exp(transT) (stationary CRF matrix, lhsT form), also used for PSTOP col
        pt_sb = consts.tile([NT, NT], FP32)
        nc.scalar.activation(pt_sb[:], transT_sb[:], AF.Exp)
        pts_sb = consts.tile([NT, NT], FP32)  # e^-CCENT-centered loop matrix
        negc_sb = consts.tile([NT, 1], FP32)
        nc.vector.memset(negc_sb[:], -CCENT)
        nc.scalar.activation(pts_sb[:], transT_sb[:], AF.Exp, bias=negc_sb[:, 0:1])

        # ---------- phase 1: gather -> transpose -> gx GEMMs ----------
        # embT and gxd are chunked per 512-token block so downstream deps are
        # per-chunk and the LSTM can start before all of phase 1 finishes.
        hallp = ctx.enter_context(tc.tile_pool(name="hallp", bufs=1))
        h_all = [hallp.tile([H, TOK], BF16, name=f"hall{i}") for i in range(2)]
        NCH = TOK // NCHUNK  # 12 chunks
        gxdp = tc.alloc_tile_pool(name="gxdp", bufs=1)
        gxd = [[gxdp.tile([128, 4 * NCHUNK], BF16, name=f"gxd{d}_{n}") for n in range(NCH)]
               for d in range(2)]
        embp = tc.alloc_tile_pool(name="embp", bufs=1)
        embT = [[embp.tile([k1 - k0, NCHUNK], BF16, name=f"embT{i}_{n}") for n in range(NCH)]
                for i, (k0, k1) in enumerate(KCH)]
        for n in range(NCH):
            nc.sync.dma_start(
                out=embT[2][n][E - 256:E - 254, :],
                in_=bass.AP(tensor=aux.tensor, offset=aux.offset + n * NCHUNK,
                            ap=[[TOK, 2], [1, NCHUNK]]),
            )

        # Phase-1 production is emitted as micro-slices interleaved into the
        # LSTM loop below: engine instruction streams are in-order, so overlap
        # only happens if producer/consumer instructions interleave in emission.
        def chunk_order():
            lo, hi = 0, NCH - 1
            out = []
            while lo <= hi:
                out.append((1, hi)); hi -= 1
                if lo <= hi:
                    out.append((0, lo)); lo += 1
            return out

        gpool = tc.alloc_tile_pool(name="gather", bufs=4)
        tpsum = tc.alloc_tile_pool(name="tpsum", bufs=2, space="PSUM")
        gxp = tc.alloc_tile_pool(name="gxp", bufs=2, space="PSUM")
        gathered = set()

        def emit_gather(c, n):
            embg = gpool.tile([128, E], BF16, name=f"embg{c}", tag="embg")
            nc.gpsimd.indirect_dma_start(
                out=embg[:], out_offset=None, in_=embtab[:],
                in_offset=bass.IndirectOffsetOnAxis(ap=sent_sb[:, c:c + 1], axis=0),
            )
            cc = (c % 4) * 128
            for ki, (k0, k1) in enumerate(KCH):
                kw = min(k1, E) - k0
                tp = tpsum.tile([128, 128], BF16, name=f"tp{c}_{ki}", tag="tp")
                nc.tensor.transpose(tp[0:kw, :], embg[:, k0:k0 + kw], ident_sb[:])
                if (c + ki) % 2 == 0:
                    nc.scalar.copy(embT[ki][n][0:kw, cc:cc + 128], tp[0:kw, :])
                else:
                    nc.vector.tensor_copy(embT[ki][n][0:kw, cc:cc + 128], tp[0:kw, :])

        def emit_gx(dd, n, g):
            mm = gxp.tile([128, NCHUNK], FP32, name=f"gxmm{dd}_{n}_{g}", tag="gxmm")
            for ki, (k0, k1) in enumerate(KCH):
                nc.tensor.matmul(
                    mm[:],
                    waug_sb[ki][:, dd * G4 + g * H: dd * G4 + (g + 1) * H],
                    embT[ki][n][:],
                    start=(ki == 0), stop=(ki == 2),
                )
            dst = gxd[dd][n][:, g * NCHUNK:(g + 1) * NCHUNK]
            if (n + g) % 2 == 0:
                nc.scalar.copy(dst, mm[:])
            else:
                nc.vector.tensor_copy(dst, mm[:])

        def production_items():
            for d, n in chunk_order():
                for c in range(4 * n, 4 * n + 4):
                    if c not in gathered:
                        gathered.add(c)
                        yield ("gather", c, n)
                for dd in (d, 1 - d):
                    for g in range(4):
                        yield ("gx", dd, n, g)

        prod = production_items()

        def emit_items(k):
            for _ in range(k):
                it = next(prod, None)
                if it is None:
                    return
                if it[0] == "gather":
                    emit_gather(it[1], it[2])
                else:
                    emit_gx(it[1], it[2], it[3])

        emit_items(24)  # first chunk of each direction up front

        # ---------- phase 2: LSTM passes ----------
        do_rest = PHASE_LIMIT >= 2
        hinit = consts.tile([H, BC], BF16)
        nc.vector.memset(hinit[:], 0.0)
        cinit = consts.tile([H, BC], FP32)
        nc.vector.memset(cinit[:], 0.0)

        lpools = {
            "gates": tc.alloc_tile_pool(name="lgates", bufs=2, space="PSUM"),
            "tio": tc.alloc_tile_pool(name="ltio", bufs=2),
            "tg": tc.alloc_tile_pool(name="ltg", bufs=2),
            "sig": tc.alloc_tile_pool(name="lsig", bufs=2),
            "ab": tc.alloc_tile_pool(name="lab", bufs=2),
            "c": tc.alloc_tile_pool(name="lc", bufs=2),
            "tc": tc.alloc_tile_pool(name="ltc", bufs=2),
        }

        def lstm_step(d, s, t, h_prev_ap, c_prev_ap):
            n, toff = t // 16, (t % 16) * BC
            gates = lpools["gates"].tile([H, 4 * BC], FP32, tag=f"gates{d}", name=f"gates{d}")
            gxt = gxd[d][n]
            gx_rhs = bass.AP(
                tensor=gxt.tensor, offset=gxt[:, toff].offset,
                ap=[gxt.ap[0], [NCHUNK, 4], [1, BC]],
            )
            # gx -> PSUM via identity matmul (PE slack; keeps ACT/DVE free)
            nc.tensor.matmul(gates[:], ident_sb[:], gx_rhs, start=True, stop=False,
                             skip_group_check=True)
            for g in range(4):
                nc.tensor.matmul(
                    gates[:, g * BC:(g + 1) * BC],
                    whh_sb[:, d * G4 + g * H: d * G4 + (g + 1) * H],
                    h_prev_ap,
                    start=False, stop=True, skip_group_check=True,
                )
            tio = lpools["tio"].tile([H, 4 * BC], FP32, tag=f"tio{d}", name=f"tio{d}")
            nc.scalar.activation(tio[:], gates[:], AF.Tanh, scale=0.5)
            sig = lpools["sig"].tile([H, 3 * BC], FP32, tag=f"sig{d}", name=f"sig{d}")
            nc.vector.tensor_scalar(sig[:], tio[:, 0:3 * BC], 0.5, 0.5, ALU.mult, ALU.add)
            a_ = lpools["ab"].tile([H, BC], FP32, tag=f"a{d}", name=f"a{d}")
            nc.vector.tensor_tensor(a_[:], sig[:, 0:BC], tio[:, 3 * BC:4 * BC], ALU.mult)
            b_ = lpools["ab"].tile([H, BC], FP32, tag=f"b{d}", name=f"b{d}")
            nc.vector.tensor_tensor(b_[:], sig[:, BC:2 * BC], c_prev_ap, ALU.mult)
            c_ = lpools["c"].tile([H, BC], FP32, tag=f"c{d}", name=f"c{d}")
            nc.vector.tensor_tensor(c_[:], a_[:], b_[:], ALU.add)
            tc_ = lpools["tc"].tile([H, BC], FP32, tag=f"tc{d}", name=f"tc{d}")
            nc.scalar.activation(tc_[:], c_[:], AF.Tanh)
            hdst = h_all[d][:, t * BC:(t + 1) * BC]
            nc.vector.tensor_tensor(hdst, sig[:, 2 * BC:3 * BC], tc_[:], ALU.mult)
            return hdst, c_[:]

        if do_rest:
            # both directions interleaved: two independent dependency chains
            hp0, cp0 = hinit[:], cinit[:]
            hp1, cp1 = hinit[:], cinit[:]
            for s in range(T):
                emit_items(2)
                hp0, cp0 = lstm_step(0, s, s, hp0, cp0)
                hp1, cp1 = lstm_step(1, s, T - 1 - s, hp1, cp1)

        for pname in ["tc", "c", "ab", "sig", "tg", "tio"]:
            lpools[pname].release()
        lpools["gates"].release()
        gxp.release()
        tpsum.release()
        gpool.release()
        embp.release()
        gxdp.release()

        # ---------- phase 3: fc GEMM -> feats -> ef ----------
        late = ctx.enter_context(tc.tile_pool(name="late", bufs=1))
        feats = late.tile([NT, TOK], BF16)
        ef = late.tile([NT, TOK], BF16)
        m13_sb = late.tile([NT, TOK], BF16)
        nc.sync.dma_start(
            out=m13_sb[:],
            in_=bass.AP(tensor=m1.tensor, offset=m1.offset, ap=[[0, NT]] + m1.ap),
        )
        oh_sb = late.tile([NT, TOK], BF16)
        nc.sync.dma_start(out=oh_sb[:], in_=oh[:])
        fcp = tc.alloc_tile_pool(name="fcp", bufs=3, space="PSUM")
        fctmp = ctx.enter_context(tc.tile_pool(name="fctmp", bufs=3))
        for n in range(TOK // NCHUNK if PHASE_LIMIT >= 3 else 0):
            mm = fcp.tile([NT, NCHUNK], FP32)
            cols = slice(n * NCHUNK, (n + 1) * NCHUNK)
            nc.tensor.matmul(mm[:], wfc_sb[:, 0:NT], h_all[0][:, cols], start=True, stop=False)
            nc.tensor.matmul(mm[:], wfc_sb[:, NT:2 * NT], h_all[1][:, cols], start=False, stop=True)
            msk = fctmp.tile([NT, NCHUNK], FP32)
            nc.vector.tensor_tensor(msk[:], mm[:], m13_sb[:, cols], ALU.mult)
            nc.scalar.activation(feats[:, cols], msk[:], AF.Identity, bias=bfc_sb[:, 0:1])
            nc.scalar.activation(ef[:, cols], msk[:], AF.Exp, bias=bfc_sb[:, 0:1])

        # ---------- phase 4: CRF forward recursion ----------
        # 2 independent batch streams; renorm folded into the NEXT step's ef
        # slice so the critical chain stays MM -> TT.
        fcp.release()
        NS = 2
        SB = BC // NS
        apool = ctx.enter_context(tc.tile_pool(name="apool", bufs=3))
        crfp = tc.alloc_tile_pool(name="crfp", bufs=2, space="PSUM")
        crfz = tc.alloc_tile_pool(name="crfz", bufs=1, space="PSUM")
        zbuf = late.tile([1, NREN * BC], FP32)
        zr = ctx.enter_context(tc.tile_pool(name="zr", bufs=2))
        efx = ctx.enter_context(tc.tile_pool(name="efx", bufs=2))
        As = []
        for s_ in range(NS):
            A0 = apool.tile([NT, SB], FP32, tag=f"A{s_}", name=f"A{s_}")
            nc.vector.memset(A0[:], 0.0)
            nc.vector.memset(A0[START:START + 1, :], 1.0)
            As.append(A0)
        pend = [None] * NS
        for t in range(T if PHASE_LIMIT >= 4 else 0):
            for s_ in range(NS):
                cols = slice(t * BC + s_ * SB, t * BC + (s_ + 1) * SB)
                r = crfp.tile([NT, SB], FP32, tag=f"r{s_}", name=f"r{s_}")
                nc.tensor.matmul(r[:], pts_sb[:], As[s_][:], start=True, stop=True)
                A2 = apool.tile([NT, SB], FP32, tag=f"A{s_}", name=f"A{s_}")
                ef_ap = pend[s_][:] if pend[s_] is not None else ef[:, cols]
                pend[s_] = None
                nc.vector.tensor_tensor(A2[:], r[:], ef_ap, ALU.mult)
                As[s_] = A2
                if (t + 1) % RENORM == 0 and t < T - RENORM:
                    k = (t + 1) // RENORM - 1
                    zrow = crfz.tile([1, SB], FP32, tag=f"zrow{s_}", name=f"zrow{s_}")
                    nc.tensor.matmul(zrow[:], ones13_sb[:], A2[:], start=True, stop=True)
                    nc.scalar.copy(zbuf[:, k * BC + s_ * SB: k * BC + (s_ + 1) * SB], zrow[:])
                    zrec = zr.tile([1, SB], FP32, tag=f"zrec{s_}", name=f"zrec{s_}")
                    nc.vector.reciprocal(zrec[:], zrow[:])
                    zbc = crfz.tile([NT, SB], FP32, tag=f"zbc{s_}", name=f"zbc{s_}")
                    nc.tensor.matmul(zbc[:], onesrow_sb[:], zrec[:], start=True, stop=True)
                    nxt = efx.tile([NT, SB], FP32, tag=f"efx{s_}", name=f"efx{s_}")
                    ncols = slice((t + 1) * BC + s_ * SB, (t + 1) * BC + (s_ + 1) * SB)
                    nc.vector.tensor_tensor(nxt[:], ef[:, ncols], zbc[:], ALU.mult)
                    pend[s_] = nxt
        A = apool.tile([NT, BC], FP32, name="Afin")
        for s_ in range(NS):
            nc.vector.tensor_copy(A[:, s_ * SB:(s_ + 1) * SB], As[s_][:])
        crfz.release()
        crfp.release()

        # ---------- phase 5: finals ----------
        fin = ctx.enter_context(tc.tile_pool(name="fin", bufs=1))
        if PHASE_LIMIT < 5:
            nc.sync.dma_start(out=out[:], in_=A[0:1, 0:1])
            return
        finp = ctx.enter_context(tc.tile_pool(name="finp", bufs=1, space="PSUM"))
        # emit + trans_sc accumulate into one PSUM [1, BC]
        TQ = T // 4
        emp = ctx.enter_context(tc.tile_pool(name="emp", bufs=2))
        emres = []
        for q in range(4):
            emtmp = emp.tile([NT, TQ * BC], FP32, tag="emtmp")
            cols = slice(q * TQ * BC, (q + 1) * TQ * BC)
            nc.vector.tensor_tensor(emtmp[:], feats[:, cols], oh_sb[:, cols], ALU.mult)
            emq = fin.tile([NT, BC], FP32, name=f"emq{q}", tag=f"emq{q}")
            nc.vector.reduce_sum(
                emq[:], emtmp[:].rearrange("p (t b) -> p b t", t=TQ),
                axis=mybir.AxisListType.X,
            )
            emres.append(emq)
        nc.vector.tensor_tensor(emres[0][:], emres[0][:], emres[1][:], ALU.add)
        nc.vector.tensor_tensor(emres[2][:], emres[2][:], emres[3][:], ALU.add)
        emred = fin.tile([NT, BC], FP32)
        nc.vector.tensor_tensor(emred[:], emres[0][:], emres[2][:], ALU.add)
        gold = finp.tile([1, BC], FP32)
        nc.tensor.matmul(gold[:], tf_sb[:, 0:1], pc_sb[:, 0:BC], start=True, stop=False)
        nc.tensor.matmul(gold[:], tf_sb[0:41, 1:2], pc_sb[0:41, BC:2 * BC], start=False, stop=False)
        nc.tensor.matmul(gold[:], ones13_sb[:], emred[:], start=False, stop=True)
        # fwd score: log(PSTOP . A) + sum_k ln Z_k
        fmm = finp.tile([1, BC], FP32)
        nc.tensor.matmul(fmm[:], pt_sb[:, STOP:STOP + 1], A[:], start=True, stop=True)
        lnz = fin.tile([1, NREN * BC], FP32)
        nc.scalar.activation(lnz[:], zbuf[:], AF.Ln)
        lsum = fin.tile([1, BC], FP32)
        nc.vector.reduce_sum(
            lsum[:], lnz[:].rearrange("p (k b) -> p b k", k=NREN), axis=mybir.AxisListType.X,
        )
        lfin = fin.tile([1, BC], FP32)
        nc.scalar.activation(lfin[:], fmm[:], AF.Ln)
        fwd = fin.tile([1, BC], FP32)
        nc.vector.tensor_tensor(fwd[:], lfin[:], lsum[:], ALU.add)
        nll = fin.tile([1, BC], FP32)
        nc.vector.tensor_tensor(nll[:], fwd[:], gold[:], ALU.subtract)
        nllc = fin.tile([1, BC], FP32)
        nc.vector.tensor_scalar_add(nllc[:], nll[:], CCENT * T)
        tot = fin.tile([1, 1], FP32)
        nc.vector.reduce_sum(tot[:], nllc[:], axis=mybir.AxisListType.X)
        nc.sync.dma_start(out=out[:], in_=tot[:])


def build_program():
    key = ("nc", PHASE_LIMIT)
    if key in _PROGRAM_CACHE:
        return _PROGRAM_CACHE[key]
    nc = bacc.Bacc("TRN2", target_bir_lowering=False, debug=False, num_devices=NCORES)
    io = {
        "ident": nc.dram_tensor("ident", [128, 128], BF16, kind="ExternalInput").ap(),
        "sent": nc.dram_tensor("sent", [TOK], I32, kind="ExternalInput").ap(),
        "embtab": nc.dram_tensor("embtab", [V, E], BF16, kind="ExternalInput").ap(),
        "waug": nc.dram_tensor("waug", [KAUG, 2 * G4], BF16, kind="ExternalInput").ap(),
        "whh": nc.dram_tensor("whh", [H, 2 * G4], BF16, kind="ExternalInput").ap(),
        "wfc": nc.dram_tensor("wfc", [H2, NT], BF16, kind="ExternalInput").ap(),
        "bfc": nc.dram_tensor("bfc", [NT, 1], FP32, kind="ExternalInput").ap(),
        "transT": nc.dram_tensor("transT", [NT, NT], FP32, kind="ExternalInput").ap(),
        "transflat": nc.dram_tensor("transflat", [NT * NT], FP32, kind="ExternalInput").ap(),
        "m1": nc.dram_tensor("m1", [TOK], BF16, kind="ExternalInput").ap(),
        "aux": nc.dram_tensor("aux", [2, TOK], BF16, kind="ExternalInput").ap(),
        "oh": nc.dram_tensor("oh", [NT, TOK], BF16, kind="ExternalInput").ap(),
        "pc": nc.dram_tensor("pc", [NT * NT, BC], FP32, kind="ExternalInput").ap(),
        "out": nc.dram_tensor("out", [1, 1], FP32, kind="ExternalOutput").ap(),
    }
    with tile.TileContext(nc) as tc:
        _emit(tc, io)
    nc.compile()
    _PROGRAM_CACHE[key] = nc
    return nc


def host_prep(inputs):
    """Build the 8 per-core input maps (host does only index/layout/dtype prep)."""
    import ml_dtypes
    bf16 = ml_dtypes.bfloat16

    sent = np.asarray(inputs["sentence"]).astype(np.int32)      # [B,T]
    seq_len = np.asarray(inputs["seq_len"]).astype(np.int64)
    tags = np.asarray(inputs["tags"]).astype(np.int64)          # [B,T]
    lens = np.clip(seq_len, 1, T)
    mask = (np.arange(T)[None, :] < lens[:, None]).astype(np.float32)  # [B,T]
    embtab_bf = np.ascontiguousarray(np.asarray(inputs["embedding"], np.float32).astype(bf16))

    def reorder(Wx):  # pytorch gate order i,f,g,o -> i,f,o,g
        i, f, g, o = np.split(np.asarray(Wx, np.float32), 4, 0)
        return np.concatenate([i, f, o, g], 0)

    def build_waug(W_ih, bvec, is_bwd):
        Wr = reorder(W_ih).copy()   # [4H, E]
        br = reorder(np.asarray(bvec, np.float32)[:, None])[:, 0].copy()
        Wr[3 * H:4 * H] *= 2.0      # g-gate preact x2: tanh(0.5*(2x)) = tanh(x)
        br[3 * H:4 * H] *= 2.0
        Waug = np.zeros((KAUG, G4), np.float32)
        Waug[0:E, :] = Wr.T
        Waug[E, :] = br             # ones row -> bias
        if is_bwd:
            Waug[E + 1, 0:2 * H] = -1e9  # (1-m) row -> i,f preact mask
        return Waug

    waug = np.concatenate(
        [build_waug(inputs["W_ih_f"], inputs["b_f"], False),
         build_waug(inputs["W_ih_b"], inputs["b_b"], True)], axis=1
    ).astype(bf16)                                               # [KAUG, 1024]
    def whh_prep(W):
        Wr = reorder(W).copy()
        Wr[3 * H:4 * H] *= 2.0
        return Wr.T
    whh = np.concatenate(
        [whh_prep(inputs["W_hh_f"]), whh_prep(inputs["W_hh_b"])], axis=1
    ).astype(np.float32).astype(bf16)                            # [H, 1024]
    wfc = np.ascontiguousarray(np.asarray(inputs["W_fc"], np.float32).T).astype(bf16)  # [H2,NT]
    bfc = np.asarray(inputs["b_fc"], np.float32).reshape(NT, 1)
    trans = np.asarray(inputs["transitions"], np.float32)
    transT = np.ascontiguousarray(trans.T)
    transflat = np.ascontiguousarray(trans.reshape(-1))
    ident = np.eye(128, dtype=np.float32).astype(bf16)

    in_maps = []
    for core in range(NCORES):
        sl = slice(core * BC, (core + 1) * BC)
        s_c, t_c, m_c = sent[sl], tags[sl], mask[sl]             # [BC,T]
        sent_tm = np.ascontiguousarray(s_c.T.reshape(-1)).astype(np.int32)   # tok=t*BC+b
        m_tm = np.ascontiguousarray(m_c.T.reshape(-1)).astype(bf16)
        aux_tm = np.stack([np.ones(TOK, np.float32),
                           1.0 - m_tm.astype(np.float32)]).astype(bf16)
        # one-hot [NT, TOK]
        ohm = np.zeros((NT, TOK), np.float32)
        ttm = t_c.T.reshape(-1)                                  # [TOK]
        ohm[ttm, np.arange(TOK)] = 1.0
        ohm = ohm.astype(bf16)
        # pair-count histogram [169, BC] incl STOP term
        pcm = np.zeros((NT * NT, BC), np.float32)
        text = np.concatenate([np.full((BC, 1), START, np.int64), t_c], 1)
        for b_ in range(BC):
            idx = text[b_, 1:] * NT + text[b_, :-1]
            np.add.at(pcm[:, b_], idx, 1.0)
            pcm[STOP * NT + t_c[b_, -1], b_] += 1.0
        in_maps.append({
            "ident": ident, "sent": sent_tm, "embtab": embtab_bf,
            "waug": waug, "whh": whh, "wfc": wfc, "bfc": bfc,
            "transT": transT, "transflat": transflat,
            "m1": m_tm, "aux": aux_tm, "oh": ohm, "pc": pcm,
        })
    return in_maps


def kernel(**inputs):
    nc = build_program()
    in_maps = host_prep(inputs)
    res = bass_utils.run_bass_kernel_spmd(nc, in_maps, list(range(NCORES)))
    total = sum(float(r["out"][0, 0]) for r in res.results)
    return np.float32(total / B)
